# revision 17
# baseline (speedup 1.0000x reference)
"""Trainium2 Bass kernel for nn_MetaLayer_2551210573871 (dense_mlp).

Math:  out[b,o] = sum_i feature[b,i] * ((signal @ T_1).reshape(B,I,O)[b,i,o] + M_1[i,o])
             = sum_{s,i} signal[b,s]*feature[b,i]*T_1[s,i,o]  +  (feature @ M_1)[b,o]

Strategy (data-parallel over batch, 8 cores, B_local = 512):
  For each s-pair p: G = feature_local @ T_1[2p:2p+2]  (PE, bf16 operands)
  out_local = sum_s signal_local[:, s] * G_s + feature_local @ M_1

vs the 185.7us f32r baseline (151us here, all numbers measured on HW):
  - all matmul operands bf16 (same 1 cyc/col PE rate as f32r, half DMA/SBUF;
    LDWEIGHTS ~97ns hides behind 216ns matmuls); fp8-e4m3 DoubleRow was
    measured at only 2x FLOPs/cycle, which accuracy-compensation eats, so bf16
    is the right dtype
  - featT/T1 layouts prepped on host (no on-chip transposes); full T1 resident
    in SBUF (16.8 MB), DMA'd once in use order
  - stage B split by measured in-situ costs; pair-outer/bt-inner keeps 4
    independent accumulation chains live per engine:
      path i (55%): 2x DVE scalar_tensor_tensor f32 PSUM -> accA/accB[bt]
      path ii (45%): 2x ACT scaled-copy ->bf16 tmp + GPS bf16 1024-wide add
    ii-units are smoothed to 1-2 per pair (a clump overruns the ACT/GPS
    pipeline, fills PSUM, stalls the PE and drops its DVFS p-state)
  - finale ordering matters: tail pairs' STTs first, then bf16 subacc flushes,
    then merge + store, so the in-order DVE queue never head-of-line blocks
    on GPS; bf16 subacc rounding adds ~5e-4 to the 2.9e-3 bf16-matmul error,
    far under the 2e-2 gate.
"""
import numpy as np
import ml_dtypes

import concourse.bacc as bacc
import concourse.mybir as mybir
import concourse.tile as tile
from concourse.bass_utils import run_bass_kernel_spmd

S_DIM, IN_DIM, OUT_DIM, BATCH = 128, 256, 256, 4096
N_CORES = 8
BL = BATCH // N_CORES          # 512 examples per core
NBT = BL // 128                # 4 batch tiles of 128
NPAIR = S_DIM // 2             # 64 s-pairs
FLUSH_EVERY = 32               # flush period per sub-accumulator (in ii-pairs)

F32 = mybir.dt.float32
BF16 = mybir.dt.bfloat16
ALU = mybir.AluOpType
ACTF = mybir.ActivationFunctionType

# stage-B path fractions per (pair,bt) unit, from measured in-situ costs:
#   i: DVE 2x394   ii: ACT 2x488 + half a GPS 1024-wide add (~1500/2 pairs)
_PATH_FRACS = {"i": 0.55, "ii": 0.45}


def _make_assignment(n_units):
    # Smooth ii placement: per pair exactly 1 or 2 ii-units (cumulative
    # tracking of the target fraction), staggered across bt so the ACT/GPS
    # pipeline never sees a clump; last 3 pairs all path-i (tail drain).
    frac_ii = _PATH_FRACS["ii"]
    per_pair = frac_ii * NBT
    out = ["i"] * n_units
    acc = 0.0
    for p in range(NPAIR - 3):
        prev = round(acc)
        acc += per_pair
        k = round(acc) - prev          # ii-units this pair (1 or 2)
        for j in range(k):
            bt = (p + j * 2) % NBT
            out[p * NBT + bt] = "ii"
    return out


def _build():
    nc = bacc.Bacc("TRN2", target_bir_lowering=False, debug=False, num_devices=N_CORES)

    sig_d = nc.dram_tensor("sig", [BL, S_DIM], F32, kind="ExternalInput")
    featT_d = nc.dram_tensor("featT", [128, 2 * BL], BF16, kind="ExternalInput")
    t1_d = nc.dram_tensor("t1", [128, 2 * NPAIR * 512], BF16, kind="ExternalInput")
    m1_d = nc.dram_tensor("m1", [IN_DIM, OUT_DIM], BF16, kind="ExternalInput")
    out_d = nc.dram_tensor("out", [BL, OUT_DIM], F32, kind="ExternalOutput")

    with tile.TileContext(nc) as tc:
        assignment = _make_assignment(NPAIR * NBT)
        with (
            tc.tile_pool(name="const", bufs=1) as const,
            tc.tile_pool(name="tmp", bufs=12) as tmp_pool,
            tc.tile_pool(name="psum", bufs=8, space="PSUM") as psum,
        ):
            # --- persistent inputs, DMA'd in first-use order with the
            # featT tile split so the first matmuls gate on ~200KB, not 640KB
            featT = const.tile([128, 2 * BL], BF16, tag="featT", name="featT")
            for bt in range(NBT):
                for ic in range(2):
                    lo = ic * BL + bt * 128
                    nc.sync.dma_start(out=featT[:, lo:lo + 128],
                                      in_=featT_d[:, lo:lo + 128])
                if bt == 0:
                    m1 = []
                    for ic in range(2):
                        t = const.tile([128, OUT_DIM], BF16, tag=f"m1_{ic}",
                                       name=f"m1_{ic}")
                        nc.sync.dma_start(
                            out=t[:], in_=m1_d[ic * 128:(ic + 1) * 128, :])
                        m1.append(t)

            def load_t1(p, t1t):
                for ic in range(2):
                    t = const.tile([128, 512], BF16, tag=f"t1_{ic}_{p}",
                                   name=f"t1_{ic}_{p}")
                    base = (ic * NPAIR + p) * 512
                    nc.sync.dma_start(out=t[:], in_=t1_d[:, base:base + 512])
                    t1t[(ic, p)] = t

            t1t = {}
            for p in range(3):
                load_t1(p, t1t)

            sig = []
            for bt in range(NBT):
                t = const.tile([128, S_DIM], F32, tag=f"sig{bt}", name=f"sig{bt}")
                nc.sync.dma_start(out=t[:], in_=sig_d[bt * 128:(bt + 1) * 128, :])
                sig.append(t)

            for p in range(3, NPAIR):
                load_t1(p, t1t)

            # preload ACT function table while DMAs stream
            warm = const.tile([128, 8], F32, tag="warm", name="warm")
            nc.scalar.activation(warm[:], sig[0][:, 0:8], ACTF.Copy,
                                 bias=0.0, scale=1.0)

            # --- accumulators ---
            accA, accB, subG = [], [], []
            for bt in range(NBT):
                accA.append(const.tile([128, OUT_DIM], F32, tag=f"accA{bt}",
                                       name=f"accA{bt}"))
                accB.append(const.tile([128, OUT_DIM], F32, tag=f"accB{bt}",
                                       name=f"accB{bt}"))
                subG.append(const.tile([128, 1024], BF16, tag=f"subG{bt}",
                                       name=f"subG{bt}"))

            # accA[bt] = feature @ M_1 ; accB[bt] = 0
            for bt in range(NBT):
                ps = psum.tile([128, 512], F32, tag="G", name="ps_init")
                for ic in range(2):
                    nc.tensor.matmul(
                        ps[:, 0:OUT_DIM],
                        featT[:, ic * BL + bt * 128:ic * BL + (bt + 1) * 128],
                        m1[ic][:],
                        start=(ic == 0),
                        stop=(ic == 1),
                    )
                nc.vector.tensor_copy(accA[bt][:], ps[:, 0:OUT_DIM])
                nc.gpsimd.memset(accB[bt][:], 0)
                nc.gpsimd.memset(subG[bt][:], 0)

            subG_n = [0] * NBT    # ii-pairs since last flush (0 => fresh)
            pend = [None] * NBT   # pending half-filled double-tmp tile

            def flush(bt, reset=True):
                for q in range(4):
                    acc = accA[bt] if q % 2 == 0 else accB[bt]
                    nc.vector.tensor_tensor(
                        acc[:], subG[bt][:, q * 256:q * 256 + 256],
                        acc[:], ALU.add)
                if reset:
                    nc.gpsimd.memset(subG[bt][:], 0)
                subG_n[bt] = 0

            # statically known last ii-unit per bt -> flush a few pairs later
            # (so the in-order DVE queue never head-of-line blocks on GPS)
            last_ii_p = [-1] * NBT
            for p_ in range(NPAIR):
                for bt_ in range(NBT):
                    if assignment[p_ * NBT + bt_] == "ii":
                        last_ii_p[bt_] = p_

            # --- main loop: pair-outer, bt-inner ---
            for p in range(NPAIR):
                g = [psum.tile([128, 512], F32, tag="G", name=f"g{bt}")
                     for bt in range(NBT)]
                for ic in range(2):
                    for bt in range(NBT):
                        nc.tensor.matmul(
                            g[bt][:],
                            featT[:, ic * BL + bt * 128:ic * BL + (bt + 1) * 128],
                            t1t[(ic, p)][:],
                            start=(ic == 0), stop=(ic == 1),
                        )
                s0, s1 = 2 * p, 2 * p + 1
                for bt in range(NBT):
                    mode = assignment[p * NBT + bt]
                    if mode == "i":
                        if p >= NPAIR - 3:
                            acc = accA[bt]
                        else:
                            acc = accA[bt] if p % 2 == 0 else accB[bt]
                        for half, s in ((0, s0), (1, s1)):
                            nc.vector.scalar_tensor_tensor(
                                acc[:], g[bt][:, half * 256:half * 256 + 256],
                                sig[bt][:, s:s + 1], acc[:],
                                ALU.mult, ALU.add,
                            )
                    else:
                        if pend[bt] is None:
                            pend[bt] = tmp_pool.tile([128, 1024], BF16,
                                                     tag="tmp", name="tmp")
                            off = 0
                        else:
                            off = 512
                        tmp = pend[bt]
                        for half, s in ((0, s0), (1, s1)):
                            nc.scalar.activation(
                                tmp[:, off + half * 256:off + half * 256 + 256],
                                g[bt][:, half * 256:half * 256 + 256],
                                ACTF.Copy, bias=0.0,
                                scale=sig[bt][:, s:s + 1],
                            )
                        if off == 512:
                            nc.gpsimd.tensor_tensor(
                                subG[bt][:], tmp[:], subG[bt][:], ALU.add)
                            pend[bt] = None
                            subG_n[bt] += 2
                            if subG_n[bt] >= FLUSH_EVERY:
                                flush(bt)
                    if p == last_ii_p[bt] and pend[bt] is not None:
                        nc.gpsimd.tensor_tensor(
                            subG[bt][:, 0:512], pend[bt][:, 0:512],
                            subG[bt][:, 0:512], ALU.add)
                        pend[bt] = None
                        subG_n[bt] += 1

            # --- finale per bt: flush bf16 subacc, merge, store ---
            for bt in range(NBT):
                assert pend[bt] is None
                if subG_n[bt]:
                    flush(bt, reset=False)
                nc.vector.tensor_tensor(accA[bt][:], accB[bt][:], accA[bt][:],
                                        ALU.add)
                nc.sync.dma_start(
                    out=out_d[bt * 128:(bt + 1) * 128, :], in_=accA[bt][:]
                )

    nc.compile()
    return nc


_cached = None


def make_in_maps(signal, feature, T_1, M_1):
    signal = np.ascontiguousarray(np.asarray(signal, dtype=np.float32))
    feature = np.asarray(feature, dtype=np.float32)
    M_1bf = np.ascontiguousarray(
        np.asarray(M_1, dtype=np.float32).astype(ml_dtypes.bfloat16))
    # T1 [s,i,o] -> [k, ic, p, half, o]: tile (ic,p) = [128, 512] with cols
    # [s=2p: o | s=2p+1: o], bf16
    T1bf = np.ascontiguousarray(
        np.asarray(T_1, dtype=np.float32)
        .reshape(NPAIR, 2, 2, 128, OUT_DIM)       # [p, half, ic, k, o]
        .transpose(3, 2, 0, 1, 4)                 # [k, ic, p, half, o]
        .reshape(128, 2 * NPAIR * 512)
        .astype(ml_dtypes.bfloat16))
    in_maps = []
    for c in range(N_CORES):
        sl = slice(c * BL, (c + 1) * BL)
        feat_l = feature[sl]                      # [BL, 256]
        featT = np.ascontiguousarray(
            feat_l.T.reshape(2, 128, BL)          # [ic, k, b]
            .transpose(1, 0, 2)                   # [k, ic, b]
            .reshape(128, 2 * BL)
            .astype(ml_dtypes.bfloat16))
        in_maps.append({
            "sig": signal[sl],
            "featT": featT,
            "t1": T1bf,
            "m1": M_1bf,
        })
    return in_maps


def kernel(signal, feature, T_1, M_1):
    global _cached
    if _cached is None:
        _cached = _build()
    nc = _cached
    in_maps = make_in_maps(signal, feature, T_1, M_1)
    res = run_bass_kernel_spmd(nc, in_maps, list(range(N_CORES))).results
    return np.concatenate([res[c]["out"] for c in range(N_CORES)], axis=0)

